# revision 52
# baseline (speedup 1.0000x reference)
"""GQA attention (16 Q heads / 4 KV heads, RoPE, n=2048, d=64) on 8 trn2 cores.

Sharding: core c = (batch b=c//4, kv-group j=c%4). Each core owns 4 query
heads sharing one KV head, computes its partial output projection
(O_heads @ Wo_rows), and the host sums the 4 partials per batch.

Pipeline per core (bf16 matmuls, fp32 PSUM accumulation):
  x DMA'd ch-major so the KV projection starts early; projections run as a
  dense PE prologue, then a software-pipelined attention stream of
  (q-chunk, head, key-block) units paced by the EXP activations on ACT
  (~1.11us per [128,1024] tile; the stream runs ~97% exp-dense).
  S^T = K^T Q bf16 (128-partition contraction, zero-padded rows -- 64-row
    matmuls are much slower on HW); P = exp on ACT; O^T+denom from matmul
    with V_aug (ones column -> denominator).
  Normalize/output-projection work is spliced between stream units in
  small pieces (single norm units, pair-split out_proj halves two units
  apart) so no splice exceeds the exp backlog.
  Tail (last round, queries 1024:2048): both denominator halves packed
  into one tile for a single reciprocal; the first output chunks' pair0
  matmuls run on the PE under the reciprocal; PSUM->SBUF staging
  alternates DVE/ACT (ACT is idle in the tail); fp16 output on the
  sync+gpsimd trigger queues (never ACT mid-stream -- a waiting DMA
  trigger there head-of-line blocks the exps).
"""

import os
import sys
import functools

import numpy as np

sys.path.insert(0, "/opt/trn_rl_repo")

import concourse.bass as bass  # noqa: E402
import concourse.bacc as bacc  # noqa: E402
import concourse.tile as tile  # noqa: E402
import concourse.mybir as mybir  # noqa: E402
from concourse.masks import make_identity  # noqa: E402

F32 = mybir.dt.float32
BF16 = mybir.dt.bfloat16
F16 = mybir.dt.float16
FP8 = mybir.dt.float8e4
EXP = mybir.ActivationFunctionType.Exp
DROW = mybir.MatmulPerfMode.DoubleRow

B, N, DIM = 2, 2048, 1024
HEADS, KVH, D = 16, 4, 64
HPC = HEADS // KVH          # q heads per core = 4
SCALE = D ** -0.5           # 1/8
QTOT = HPC * N              # 8192 concatenated query columns
NKB = N // 128              # 16 key blocks
NDB = DIM // 128            # 8 contraction blocks for projections

USE_FP8 = os.environ.get("KERNEL_FP8") == "1"

LAST_RESULTS = {}           # test.py introspection


def _build(nc, tc, io):
    from contextlib import ExitStack

    xt, wq, wkv, wo = io["xt"], io["wq"], io["wkv"], io["wo"]
    cost, sincat, out = io["cost"], io["sincat"], io["out"]

    es = ExitStack()
    consts = es.enter_context(tc.tile_pool(name="consts", bufs=1))
    acts = es.enter_context(tc.tile_pool(name="acts", bufs=1))

    wq_sb = consts.tile([128, NDB, 256], BF16, tag="wq")
    wkv_sb = consts.tile([128, NDB, 128], BF16, tag="wkv")
    wo_sb = consts.tile([128, 2, DIM], BF16, tag="wo")
    cos2 = consts.tile([128, N], BF16, tag="cos2")
    sin2 = consts.tile([128, N], BF16, tag="sin2")
    id64 = consts.tile([64, 64], F32, tag="id")

    vt_sb = acts.tile([64, N], F32, tag="vt")
    vaug_sb = acts.tile([128, NKB, 65], BF16, tag="vaug")
    ot_sb = [
        acts.tile([128, N], BF16, tag=f"ot{i}", name=f"ot{i}") for i in range(2)
    ]
    if USE_FP8:
        k8 = acts.tile([32, 2, N], FP8, tag="k8")
        q8 = acts.tile([32, 2, QTOT], FP8, tag="q8")
    else:
        kt_sb = acts.tile([128, N], BF16, tag="kt")
        qt_sb = acts.tile([128, QTOT], BF16, tag="qt")

    ones64 = consts.tile([1, 64], BF16, tag="ones64")
    dpack = acts.tile([33, 512], F32, tag="dpk")

    with tc.tile_pool(name="xtp", bufs=1) as xt_pool:
        xt_sb = xt_pool.tile([128, NDB, N], BF16, tag="xt")

        # DMA issue order: KV weights + ch0 tables + ch0 x first. Early
        # triggers alternate between the two HWDGE queues (SP + Act) so the
        # ~0.6us-per-trigger dispatch doesn't serialize the lead-in.
        # x (the bulk) goes through gpsimd SWDGE triggers: ~25ns dispatch
        # each vs ~600ns on the HWDGE queues, so all 32 transfers are in
        # flight almost immediately. Weights/tables ride the two HWDGE
        # queues in priority order.
        eng = [nc.sync, nc.scalar]
        # identity first: it is tiny and the V transposes need it early,
        # while the big kt/qt memsets wait until after the x triggers.
        make_identity(nc, id64)
        # all of x through gpsimd SWDGE triggers (own queue, in flight
        # early); weights/tables split across the two HWDGE queues in
        # priority order.
        for ch in range(4):
            for kb in range(NDB):
                nc.gpsimd.dma_start(
                    xt_sb[:, kb, ch * 512:(ch + 1) * 512],
                    xt[kb, :, ch * 512:(ch + 1) * 512],
                )
        nc.vector.memset(ones64, 1.0)
        if not USE_FP8:
            nc.gpsimd.memset(kt_sb[64:128, :], 0.0)
            nc.gpsimd.memset(qt_sb[64:128, :], 0.0)
        for g in range(4):
            eng[g % 2].dma_start(
                wkv_sb[:, 2 * g:2 * g + 2, :],
                wkv[2 * g:2 * g + 2].transpose([1, 0, 2]),
            )
        eng[0].dma_start(cos2[0:64, 0:512], cost[:, 0:512])
        eng[1].dma_start(sin2[0:64, 0:512], sincat[:, 0:512])
        eng[0].dma_start(cos2[0:64, 512:1024], cost[:, 512:1024])
        eng[1].dma_start(sin2[0:64, 512:1024], sincat[:, 512:1024])
        for kb in range(NDB):
            eng[kb % 2].dma_start(wq_sb[:, kb, :], wq[kb])
        eng[0].dma_start(cos2[0:64, 1024:N], cost[:, 1024:N])
        eng[1].dma_start(sin2[0:64, 1024:N], sincat[:, 1024:N])
        eng[0].dma_start(cos2[64:128, 0:1024], cost[:, 0:1024])
        eng[1].dma_start(sin2[64:128, 0:1024], sincat[:, 0:1024])
        eng[0].dma_start(cos2[64:128, 1024:N], cost[:, 1024:N])
        eng[1].dma_start(sin2[64:128, 1024:N], sincat[:, 1024:N])
        for pair in range(2):
            for half in range(2):
                eng[half].dma_start(
                    wo_sb[:, pair, half * 512:(half + 1) * 512],
                    wo[pair][:, half * 512:(half + 1) * 512],
                )

        with (
            tc.tile_pool(name="stage", bufs=3) as stage,
            tc.tile_pool(name="rtmp", bufs=3) as rtmp,
        ):
            # ---- Q projection + rope (pipelined into the attention rounds).
            # Staging goes on ACT while it is idle (phase 1) and on DVE once
            # the EXP stream owns ACT. PSUM comes from the caller's pool:
            # phase-1 [128,512] tiles, or a wide [128,1024] attention tile.
            def q_proj_mms(ch, pack, pq, lo, hi):
                cs = slice(ch * 512, (ch + 1) * 512)
                for kb in range(lo, hi):
                    nc.tensor.matmul(
                        pq,
                        wq_sb[:, kb, pack * 128:(pack + 1) * 128],
                        xt_sb[:, kb, cs],
                        start=(kb == 0),
                        stop=(kb == NDB - 1),
                    )

            def q_rope_tail(ch, stage_act, pq, pack):
                cs = slice(ch * 512, (ch + 1) * 512)
                if True:
                    qs = stage.tile([128, 512], BF16, tag="qs")
                    if stage_act:
                        nc.scalar.copy(qs, pq)
                    else:
                        nc.vector.tensor_copy(qs, pq)
                    t1 = rtmp.tile([128, 512], BF16, tag="t1q")
                    t2 = rtmp.tile([128, 512], BF16, tag="t2q")
                    nc.vector.tensor_mul(t1, qs, cos2[:, cs])
                    for hh in range(2):
                        r = 64 * hh
                        nc.vector.tensor_mul(
                            t2[r:r + 32, :], qs[r + 32:r + 64, :],
                            sin2[r + 32:r + 64, cs],
                        )
                        nc.vector.tensor_mul(
                            t2[r + 32:r + 64, :], qs[r:r + 32, :],
                            sin2[r:r + 32, cs],
                        )
                    for hh in range(2):
                        h = pack * 2 + hh
                        r = 64 * hh
                        qcols = slice(h * N + ch * 512, h * N + (ch + 1) * 512)
                        if USE_FP8:
                            nc.vector.tensor_add(
                                q8[:, 0, qcols], t1[r:r + 32, :], t2[r:r + 32, :]
                            )
                            nc.vector.tensor_add(
                                q8[:, 1, qcols],
                                t1[r + 32:r + 64, :],
                                t2[r + 32:r + 64, :],
                            )
                        else:
                            nc.vector.tensor_add(
                                qt_sb[0:64, qcols],
                                t1[r:r + 64, :],
                                t2[r:r + 64, :],
                            )

            def q_proj_ch(ch, stage_act, psum_pool):
                for pack in range(2):
                    pq = psum_pool.tile([128, 512], F32, tag="pj",
                                        name=f"pq{ch}_{pack}")
                    q_proj_pack(ch, stage_act, pq, pack)

            # ---- attention + everything pools ----
            # PSUM: psS 3x[128,1024] (6 banks) + psO 2x[65,512] (2 banks).
            # KV projection, Q projection and the output projection all
            # borrow psS wide tiles, so the whole kernel is one stream.
            with (
                tc.tile_pool(name="ppool", bufs=6) as ppool,
                tc.tile_pool(name="unnp", bufs=10) as unnp,
                tc.tile_pool(name="den8p", bufs=5) as den8p,
                tc.tile_pool(name="rc8p", bufs=2) as rc8p,
                tc.tile_pool(name="bcp", bufs=2) as bcp,
                tc.tile_pool(name="psS", bufs=3, space="PSUM") as psS,
                tc.tile_pool(name="psO", bufs=2, space="PSUM") as psO,
                tc.tile_pool(name="ostage", bufs=4) as ostage,
            ):
                def kv_ch(ch):
                    kvt = psS.tile([128, 1024], F32, tag="s")
                    pkv = kvt[:, 0:512]
                    cs = slice(ch * 512, (ch + 1) * 512)
                    for kb in range(NDB):
                        nc.tensor.matmul(
                            pkv,
                            wkv_sb[:, kb, :],
                            xt_sb[:, kb, cs],
                            start=(kb == 0),
                            stop=(kb == NDB - 1),
                        )
                    ks = stage.tile([64, 512], BF16, tag="ks")
                    nc.scalar.copy(ks, pkv[0:64, :])
                    nc.scalar.copy(vt_sb[:, cs], pkv[64:128, :])
                    t1 = rtmp.tile([64, 512], BF16, tag="t1")
                    t2 = rtmp.tile([64, 512], BF16, tag="t2")
                    nc.vector.tensor_mul(t1, ks, cos2[0:64, cs])
                    nc.vector.tensor_mul(t2[0:32, :], ks[32:64, :], sin2[32:64, cs])
                    nc.vector.tensor_mul(t2[32:64, :], ks[0:32, :], sin2[0:32, cs])
                    if USE_FP8:
                        nc.vector.tensor_add(k8[:, 0, cs], t1[0:32, :], t2[0:32, :])
                        nc.vector.tensor_add(k8[:, 1, cs], t1[32:64, :], t2[32:64, :])
                    else:
                        nc.vector.tensor_add(kt_sb[0:64, cs], t1, t2)

                def v_trans(ch):
                    # V_aug blocks via PE transposes into a borrowed wide
                    # psS tile (fp32 path: transpose out dtype must match vt)
                    tr = psS.tile([128, 1024], F32, tag="s")
                    for i, t in enumerate(range(4 * ch, 4 * ch + 4)):
                        nc.tensor.transpose(
                            tr[:, i * 64:(i + 1) * 64],
                            vt_sb[:, t * 128:(t + 1) * 128],
                            id64,
                        )
                        nc.vector.tensor_copy(
                            vaug_sb[:, t, 0:64], tr[:, i * 64:(i + 1) * 64]
                        )
                        nc.vector.memset(vaug_sb[:, t, 64:65], 1.0)

                def q_proj_wide(ch, pack, stage_act=False):
                    pq = psS.tile([128, 1024], F32, tag="s")
                    q_proj_mms(ch, pack, pq[:, 0:512], 0, NDB)
                    q_rope_tail(ch, stage_act, pq[:, 0:512], pack)

                # prologue: projections feed the stream just-in-time; the
                # transposes trail one block so the PE covers staging latency
                nc.vector.memset(dpack, 1.0)
                kv_ch(0)
                q_proj_wide(0, 0, True)
                v_trans(0)
                kv_ch(1)
                q_proj_wide(0, 1, True)
                v_trans(1)
                kv_ch(2)
                q_proj_wide(1, 0, True)
                v_trans(2)
                kv_ch(3)
                q_proj_wide(1, 1, True)
                v_trans(3)

                # ---- software-pipelined attention stream ----
                # One global stream of (qh, h, kb) units. PV trails S by SKEW
                # units so the PE never queues behind the (pacing) EXP stream
                # at round boundaries. Normalization + output projection are
                # spliced in at PV-round completions.
                SKEW = 5
                units = [(qh, h, kb)
                         for qh in range(2) for h in range(HPC)
                         for kb in range(NKB)]
                NTILE = {0: [(0, 0), (0, 1), (0, 2), (0, 3),
                             (1, 0), (1, 1), (1, 2), (1, 3)],
                         1: [(0, 0), (0, 1), (0, 2), (0, 3),
                             (1, 0), (1, 1), (2, 0), (2, 1)]}

                dens = {}
                for qh in range(2):
                    ntiles = 1 + max(t for t, _ in NTILE[qh])
                    dens[qh] = [den8p.tile([97, 512], F32, tag="d8",
                                           name=f"den{qh}_{t}")
                                for t in range(ntiles)]
                    for t in range(ntiles):
                        nc.vector.memset(dens[qh][t], 1.0)
                unns = {0: [], 1: []}
                recs = {}
                po_of = {}
                pt_of = {}

                def emit_S(u):
                    qh, h, kb = units[u]
                    base = h * N + qh * 1024
                    ps = psS.tile([128, 1024], F32, tag="s")
                    for half in range(2):
                        qcols = slice(base + half * 512,
                                      base + (half + 1) * 512)
                        if USE_FP8:
                            nc.tensor.matmul(
                                ps[:, half * 512:(half + 1) * 512],
                                k8[:, :, kb * 128:(kb + 1) * 128],
                                q8[:, :, qcols],
                                start=True, stop=True, perf_mode=DROW,
                            )
                        else:
                            nc.tensor.matmul(
                                ps[:, half * 512:(half + 1) * 512],
                                kt_sb[:, kb * 128:(kb + 1) * 128],
                                qt_sb[:, qcols],
                                start=True, stop=True,
                            )
                    p_t = ppool.tile([128, 1024], BF16, tag="p")
                    nc.scalar.activation(p_t, ps, EXP, bias=0.0, scale=SCALE)
                    pt_of[u] = p_t

                def emit_PV(u):
                    qh, h, kb = units[u]
                    r = u // NKB
                    if kb == 0:
                        po_of[r] = [psO.tile([65, 512], F32, tag="o",
                                             name=f"po{r}_{i}")
                                    for i in range(2)]
                    po = po_of[r]
                    p_t = pt_of.pop(u)
                    for half in range(2):
                        nc.tensor.matmul(
                            po[half],
                            vaug_sb[:, kb, :],
                            p_t[:, half * 512:(half + 1) * 512],
                            start=(kb == 0),
                            stop=(kb == NKB - 1),
                            skip_group_check=True,
                        )
                    if kb == NKB - 1 and r != 7:
                        for half in range(2):
                            rr = h * 2 + half
                            unn = unnp.tile([64, 512], BF16, tag="unn",
                                            name=f"unn{qh}_{rr}")
                            nc.vector.tensor_copy(unn, po[half][0:64, :])
                            t, row = NTILE[qh][rr]
                            nc.vector.tensor_copy(
                                dens[qh][t][row * 32:row * 32 + 1, :],
                                po[half][64:65, :],
                            )
                            unns[qh].append(unn)

                def recip(qh, t):
                    rc = rc8p.tile([97, 512], F32, tag="rc8",
                                   name=f"rec{qh}_{t}")
                    nc.vector.reciprocal(rc, dens[qh][t])
                    recs[(qh, t)] = rc

                def norm_units(qh, rlist):
                    for r in rlist:
                        h, half = r // 2, r % 2
                        pair, row0 = h // 2, 64 * (h % 2)
                        t, row = NTILE[qh][r]
                        rr = rc8p.tile([1, 512], BF16, tag="rr")
                        nc.vector.tensor_copy(
                            rr, recs[(qh, t)][row * 32:row * 32 + 1, :])
                        bc = bcp.tile([64, 512], BF16, tag="bc")
                        nc.gpsimd.partition_broadcast(bc, rr)
                        ocols = slice(qh * 1024 + half * 512,
                                      qh * 1024 + (half + 1) * 512)
                        nc.vector.tensor_mul(
                            ot_sb[pair][row0:row0 + 64, ocols],
                            unns[qh][r],
                            bc,
                        )

                def out_proj(qh, qbs):
                    for qb in qbs:
                        pt = psS.tile([128, 1024], F32, tag="s")
                        for nch in range(2):
                            for pair in range(2):
                                nc.tensor.matmul(
                                    pt[:, nch * 512:(nch + 1) * 512],
                                    ot_sb[pair][:, qb * 128:(qb + 1) * 128],
                                    wo_sb[:, pair, nch * 512:(nch + 1) * 512],
                                    start=(pair == 0),
                                    stop=(pair == 1),
                                )
                        st = ostage.tile([128, 1024], F16, tag="st")
                        for nch in range(2):
                            nc.vector.tensor_copy(
                                st[:, nch * 512:(nch + 1) * 512],
                                pt[:, nch * 512:(nch + 1) * 512],
                            )
                        # two [64,1024] transfers (2KB descriptors), one per
                        # trigger queue, so the tail drains in parallel
                        for ph, e in ((0, nc.sync), (1, nc.gpsimd)):
                            e.dma_start(
                                out[qb * 128 + ph * 64:
                                    qb * 128 + (ph + 1) * 64, :],
                                st[ph * 64:(ph + 1) * 64, :],
                            )

                op_state = {}

                def op_a(qb):
                    pt = psS.tile([128, 1024], F32, tag="s")
                    op_state[qb] = pt
                    for nch in range(2):
                        nc.tensor.matmul(
                            pt[:, nch * 512:(nch + 1) * 512],
                            ot_sb[0][:, qb * 128:(qb + 1) * 128],
                            wo_sb[:, 0, nch * 512:(nch + 1) * 512],
                            start=True, stop=False,
                        )

                def op_b(qb, stage_act=False):
                    pt = op_state.pop(qb)
                    for nch in range(2):
                        nc.tensor.matmul(
                            pt[:, nch * 512:(nch + 1) * 512],
                            ot_sb[1][:, qb * 128:(qb + 1) * 128],
                            wo_sb[:, 1, nch * 512:(nch + 1) * 512],
                            start=False, stop=True,
                        )
                    st = ostage.tile([128, 1024], F16, tag="st")
                    for nch in range(2):
                        if stage_act:
                            nc.scalar.copy(
                                st[:, nch * 512:(nch + 1) * 512],
                                pt[:, nch * 512:(nch + 1) * 512],
                            )
                        else:
                            nc.vector.tensor_copy(
                                st[:, nch * 512:(nch + 1) * 512],
                                pt[:, nch * 512:(nch + 1) * 512],
                            )
                    for ph, e in ((0, nc.sync), (1, nc.gpsimd)):
                        e.dma_start(
                            out[qb * 128 + ph * 64:
                                qb * 128 + (ph + 1) * 64, :],
                            st[ph * 64:(ph + 1) * 64, :],
                        )

                # qp splices split into 4-matmul halves so the EXP
                # backlog fully hides each insertion
                qp_state = {}

                def qp_h1(ch, pack):
                    pq = psS.tile([128, 1024], F32, tag="s")
                    qp_state[(ch, pack)] = pq
                    q_proj_mms(ch, pack, pq[:, 0:512], 0, 4)

                def qp_h2(ch, pack):
                    pq = qp_state.pop((ch, pack))
                    q_proj_mms(ch, pack, pq[:, 0:512], 4, NDB)
                    q_rope_tail(ch, False, pq[:, 0:512], pack)

                def tail_norm_out():
                    # round 7 (qh1, h3): pack both halves' denominators into
                    # one tile (rows 0/32) so ONE reciprocal covers them; the
                    # first chunks' pair0 matmuls run on the PE while the
                    # reciprocal chain occupies the DVE; staging alternates
                    # DVE/ACT (ACT is idle in the tail).
                    po = po_of[7]
                    nc.vector.tensor_copy(dpack[0:1, :], po[0][64:65, :])
                    nc.vector.tensor_copy(dpack[32:33, :], po[1][64:65, :])
                    rcp = rc8p.tile([33, 512], F32, tag="rcp")
                    nc.vector.reciprocal(rcp, dpack)
                    for qb in (8, 9, 10):
                        op_a(qb)
                    for half in range(2):
                        rbT = rc8p.tile([1, 512], BF16, tag="rbt")
                        nc.vector.tensor_copy(
                            rbT, rcp[32 * half:32 * half + 1, :])
                        bcT = bcp.tile([64, 512], BF16, tag="bct")
                        nc.gpsimd.partition_broadcast(bcT, rbT)
                        oc = slice(1024 + half * 512,
                                   1024 + (half + 1) * 512)
                        nc.vector.tensor_mul(
                            ot_sb[1][64:128, oc],
                            po[half][0:64, :],
                            bcT,
                        )
                        if half == 0:
                            for i, qb in enumerate((8, 9, 10)):
                                op_b(qb, stage_act=(i % 2 == 1))
                            op_a(11)
                            op_b(11)
                        else:
                            for i, qb in enumerate((12, 13, 14, 15)):
                                op_a(qb)
                                op_b(qb, stage_act=(i % 2 == 1))

                # Splice points: small chunks of projection / normalize /
                # output work dropped between stream units so the PE never
                # inserts more work than the EXP backlog can hide.
                splice = {
                    13: [lambda: qp_h1(2, 0)],
                    15: [lambda: qp_h2(2, 0)],
                    17: [lambda: qp_h1(2, 1)],
                    19: [lambda: qp_h2(2, 1)],
                    25: [lambda: qp_h1(3, 0)],
                    27: [lambda: qp_h2(3, 0)],
                    29: [lambda: qp_h1(3, 1)],
                    31: [lambda: qp_h2(3, 1)],
                    35: [lambda: recip(0, 0)],
                    37: [lambda: norm_units(0, [0])],
                    39: [lambda: norm_units(0, [1])],
                    41: [lambda: norm_units(0, [2])],
                    43: [lambda: norm_units(0, [3])],
                    65: [lambda: recip(0, 1)],
                    66: [lambda: norm_units(0, [4])],
                    68: [lambda: norm_units(0, [5])],
                    70: [lambda: norm_units(0, [6])],
                    72: [lambda: norm_units(0, [7])],
                    74: [lambda: op_a(0)],
                    76: [lambda: op_b(0)],
                    78: [lambda: op_a(1)],
                    80: [lambda: op_b(1)],
                    82: [lambda: op_a(2)],
                    84: [lambda: op_b(2)],
                    86: [lambda: op_a(3)],
                    88: [lambda: op_b(3)],
                    90: [lambda: op_a(4)],
                    92: [lambda: op_b(4)],
                    94: [lambda: op_a(5)],
                    96: [lambda: op_b(5)],
                    97: [lambda: recip(1, 0)],
                    99: [lambda: norm_units(1, [0])],
                    101: [lambda: norm_units(1, [1])],
                    104: [lambda: norm_units(1, [2])],
                    106: [lambda: norm_units(1, [3])],
                    108: [lambda: op_a(6)],
                    110: [lambda: op_b(6)],
                    112: [lambda: op_a(7)],
                    114: [lambda: op_b(7)],
                    116: [lambda: recip(1, 1)],
                    118: [lambda: norm_units(1, [4])],
                    120: [lambda: norm_units(1, [5])],
                    127: [tail_norm_out],
                }

                NU = len(units)
                for u in range(NU + SKEW):
                    if u < NU:
                        emit_S(u)
                    v = u - SKEW
                    if v >= 0:
                        emit_PV(v)
                        for fn in splice.get(v, []):
                            fn()

    es.close()


def _rope_tables():
    inv_freq = 1.0 / (10000.0 ** (np.arange(0, D, 2, dtype=np.float64) / D))
    freqs = np.outer(np.arange(N, dtype=np.float64), inv_freq)  # [N, 32]
    cos_h = np.cos(freqs).astype(np.float32).T                  # [32, N]
    sin_h = np.sin(freqs).astype(np.float32).T                  # [32, N]
    cost = np.concatenate([cos_h, cos_h], 0)                    # [64, N]
    # sign layout [+sin; -sin]: the crossing multiplies index this table at
    # the SOURCE partitions (walrus requires equal base partitions for
    # SBUF+SBUF tensor_tensor inputs), so row r holds the sign of the row it
    # multiplies INTO the other half.
    sincat = np.concatenate([sin_h, -sin_h], 0)                 # [64, N]
    return np.ascontiguousarray(cost), np.ascontiguousarray(sincat)


@functools.lru_cache(maxsize=1)
def _program():
    nc = bacc.Bacc(
        "TRN2", target_bir_lowering=False, debug=False, enable_asserts=False
    )
    io = {
        "xt": nc.dram_tensor("xt", [NDB, 128, N], BF16, kind="ExternalInput").ap(),
        "wq": nc.dram_tensor("wq", [NDB, 128, 256], BF16, kind="ExternalInput").ap(),
        "wkv": nc.dram_tensor("wkv", [NDB, 128, 128], BF16, kind="ExternalInput").ap(),
        "wo": nc.dram_tensor("wo", [2, 128, DIM], BF16, kind="ExternalInput").ap(),
        "cost": nc.dram_tensor("cost", [64, N], BF16, kind="ExternalInput").ap(),
        "sincat": nc.dram_tensor("sincat", [64, N], BF16, kind="ExternalInput").ap(),
        "out": nc.dram_tensor("out", [N, DIM], F16, kind="ExternalOutput").ap(),
    }
    with tile.TileContext(nc) as tc:
        _build(nc, tc, io)
    nc.compile()
    return nc


def make_in_maps(x, Wq, Wkv, Wo):
    import ml_dtypes

    bf16 = ml_dtypes.bfloat16
    cost, sincat = _rope_tables()
    in_maps = []
    for c in range(8):
        b, j = c // 4, c % 4
        xt = np.ascontiguousarray(x[b].T).reshape(NDB, 128, N)
        wq_c = np.ascontiguousarray(Wq[:, 256 * j:256 * (j + 1)]).reshape(
            NDB, 128, 256
        )
        wkv_c = np.ascontiguousarray(
            np.concatenate(
                [Wkv[:, 64 * j:64 * (j + 1)],
                 Wkv[:, 256 + 64 * j:256 + 64 * (j + 1)]],
                axis=1,
            )
        ).reshape(NDB, 128, 128)
        wo_c = np.ascontiguousarray(Wo[256 * j:256 * (j + 1), :]).reshape(
            2, 128, DIM
        )
        in_maps.append(
            {
                "xt": xt.astype(bf16),
                "wq": wq_c.astype(bf16),
                "wkv": wkv_c.astype(bf16),
                "wo": wo_c.astype(bf16),
                "cost": cost.astype(bf16),
                "sincat": sincat.astype(bf16),
            }
        )
    return in_maps


def _install_ntff_hook():
    """Register the axon NTFF profiling hook that this image's antenv lacks."""
    import types

    if "antenv.axon_hooks" in sys.modules:
        return
    try:
        sys.path.append("/root/.axon_site")
        from trn_agent_boot.trn_boot import _ntff_profile_via_ctypes

        hook = _ntff_profile_via_ctypes("/opt/axon/libaxon_pjrt.so")
    except Exception:
        hook = None
    finally:
        try:
            sys.path.remove("/root/.axon_site")
        except ValueError:
            pass
    mod = types.ModuleType("antenv.axon_hooks")
    mod.get_axon_ntff_profile_hook = lambda: hook
    mod.set_axon_ntff_profile_hook = lambda h: None
    sys.modules["antenv.axon_hooks"] = mod
    # artifact upload needs bucket credentials this container lacks
    import concourse.bass_utils as bu

    bu.upload_artifacts = lambda tmpdir: "local://" + str(tmpdir)


def kernel(x, Wq, Wkv, Wo, bo):
    from concourse.bass_utils import run_bass_kernel_spmd

    _install_ntff_hook()
    nc = _program()
    in_maps = make_in_maps(x, Wq, Wkv, Wo)
    trace = bool(os.environ.get("KERNEL_TRACE"))
    res = run_bass_kernel_spmd(
        nc, in_maps, list(range(8)), trace=trace
    )
    LAST_RESULTS["res"] = res
    full = np.zeros((B, N, DIM), np.float32)
    for c in range(8):
        full[c // 4] += res.results[c]["out"].astype(np.float32)
    full += bo.astype(np.float32)
    return full



# revision 53
# speedup vs baseline: 1.1831x; 1.1831x over previous
"""GQA attention (16 Q heads / 4 KV heads, RoPE, n=2048, d=64) on 8 trn2 cores.

Sharding: core c = (batch b=c//4, kv-group j=c%4). Each core owns 4 query
heads sharing one KV head, computes its partial output projection
(O_heads @ Wo_rows), and the host sums the 4 partials per batch.

Pipeline per core (bf16 matmuls, fp32 PSUM accumulation):
  x DMA'd ch-major so the KV projection starts early; projections run as a
  dense PE prologue, then a software-pipelined attention stream of
  (q-chunk, head, key-block) units paced by the EXP activations on ACT
  (~1.11us per [128,1024] tile; the stream runs ~97% exp-dense).
  S^T = K^T Q bf16 (128-partition contraction, zero-padded rows -- 64-row
    matmuls are much slower on HW); P = exp on ACT; O^T+denom from matmul
    with V_aug (ones column -> denominator).
  Normalize/output-projection work is spliced between stream units in
  small pieces (single norm units, pair-split out_proj halves two units
  apart) so no splice exceeds the exp backlog.
  Tail (last round, queries 1024:2048): both denominator halves packed
  into one tile for a single reciprocal; the first output chunks' pair0
  matmuls run on the PE under the reciprocal; PSUM->SBUF staging
  alternates DVE/ACT (ACT is idle in the tail); fp16 output on the
  sync+gpsimd trigger queues (never ACT mid-stream -- a waiting DMA
  trigger there head-of-line blocks the exps).
"""

import os
import sys
import functools

import numpy as np

sys.path.insert(0, "/opt/trn_rl_repo")

import concourse.bass as bass  # noqa: E402
import concourse.bacc as bacc  # noqa: E402
import concourse.tile as tile  # noqa: E402
import concourse.mybir as mybir  # noqa: E402
from concourse.masks import make_identity  # noqa: E402

F32 = mybir.dt.float32
BF16 = mybir.dt.bfloat16
F16 = mybir.dt.float16
FP8 = mybir.dt.float8e4
EXP = mybir.ActivationFunctionType.Exp
DROW = mybir.MatmulPerfMode.DoubleRow

B, N, DIM = 2, 2048, 1024
HEADS, KVH, D = 16, 4, 64
HPC = HEADS // KVH          # q heads per core = 4
SCALE = D ** -0.5           # 1/8
QTOT = HPC * N              # 8192 concatenated query columns
NKB = N // 128              # 16 key blocks
NDB = DIM // 128            # 8 contraction blocks for projections

USE_FP8 = os.environ.get("KERNEL_FP8") == "1"

LAST_RESULTS = {}           # test.py introspection


def _build(nc, tc, io):
    from contextlib import ExitStack

    xt, wq, wkv, wo = io["xt"], io["wq"], io["wkv"], io["wo"]
    cost, sincat, out = io["cost"], io["sincat"], io["out"]

    es = ExitStack()
    consts = es.enter_context(tc.tile_pool(name="consts", bufs=1))
    acts = es.enter_context(tc.tile_pool(name="acts", bufs=1))

    wq_sb = consts.tile([128, NDB, 256], BF16, tag="wq")
    wkv_sb = consts.tile([128, NDB, 128], BF16, tag="wkv")
    wo_sb = consts.tile([128, 2, DIM], BF16, tag="wo")
    cos2 = consts.tile([128, N], BF16, tag="cos2")
    sin2 = consts.tile([128, N], BF16, tag="sin2")
    id64 = consts.tile([64, 64], F32, tag="id")

    vt_sb = acts.tile([64, N], F32, tag="vt")
    vaug_sb = acts.tile([128, NKB, 65], BF16, tag="vaug")
    ot_sb = [
        acts.tile([128, N], BF16, tag=f"ot{i}", name=f"ot{i}") for i in range(2)
    ]
    if USE_FP8:
        k8 = acts.tile([32, 2, N], FP8, tag="k8")
        q8 = acts.tile([32, 2, QTOT], FP8, tag="q8")
    else:
        kt_sb = acts.tile([128, N], BF16, tag="kt")
        qt_sb = acts.tile([128, QTOT], BF16, tag="qt")

    ones64 = consts.tile([1, 64], BF16, tag="ones64")
    dpack = acts.tile([33, 512], F32, tag="dpk")

    with tc.tile_pool(name="xtp", bufs=1) as xt_pool:
        xt_sb = xt_pool.tile([128, NDB, N], BF16, tag="xt")

        # DMA issue order: KV weights + ch0 tables + ch0 x first. Early
        # triggers alternate between the two HWDGE queues (SP + Act) so the
        # ~0.6us-per-trigger dispatch doesn't serialize the lead-in.
        # x (the bulk) goes through gpsimd SWDGE triggers: ~25ns dispatch
        # each vs ~600ns on the HWDGE queues, so all 32 transfers are in
        # flight almost immediately. Weights/tables ride the two HWDGE
        # queues in priority order.
        eng = [nc.sync, nc.scalar]
        # identity first: it is tiny and the V transposes need it early,
        # while the big kt/qt memsets wait until after the x triggers.
        make_identity(nc, id64)
        # all of x through gpsimd SWDGE triggers (own queue, in flight
        # early); weights/tables split across the two HWDGE queues in
        # priority order.
        for ch in range(4):
            for kb in range(NDB):
                nc.gpsimd.dma_start(
                    xt_sb[:, kb, ch * 512:(ch + 1) * 512],
                    xt[kb, :, ch * 512:(ch + 1) * 512],
                )
        nc.vector.memset(ones64, 1.0)
        if not USE_FP8:
            nc.gpsimd.memset(kt_sb[64:128, :], 0.0)
            nc.gpsimd.memset(qt_sb[64:128, :], 0.0)
        for g in range(4):
            eng[g % 2].dma_start(
                wkv_sb[:, 2 * g:2 * g + 2, :],
                wkv[2 * g:2 * g + 2].transpose([1, 0, 2]),
            )
        eng[0].dma_start(cos2[0:64, 0:512], cost[:, 0:512])
        eng[1].dma_start(sin2[0:64, 0:512], sincat[:, 0:512])
        eng[0].dma_start(cos2[0:64, 512:1024], cost[:, 512:1024])
        eng[1].dma_start(sin2[0:64, 512:1024], sincat[:, 512:1024])
        for kb in range(NDB):
            eng[kb % 2].dma_start(wq_sb[:, kb, :], wq[kb])
        eng[0].dma_start(cos2[0:64, 1024:N], cost[:, 1024:N])
        eng[1].dma_start(sin2[0:64, 1024:N], sincat[:, 1024:N])
        eng[0].dma_start(cos2[64:128, 0:1024], cost[:, 0:1024])
        eng[1].dma_start(sin2[64:128, 0:1024], sincat[:, 0:1024])
        eng[0].dma_start(cos2[64:128, 1024:N], cost[:, 1024:N])
        eng[1].dma_start(sin2[64:128, 1024:N], sincat[:, 1024:N])
        for pair in range(2):
            for half in range(2):
                eng[half].dma_start(
                    wo_sb[:, pair, half * 512:(half + 1) * 512],
                    wo[pair][:, half * 512:(half + 1) * 512],
                )

        with (
            tc.tile_pool(name="stage", bufs=3) as stage,
            tc.tile_pool(name="rtmp", bufs=3) as rtmp,
        ):
            # ---- Q projection + rope (pipelined into the attention rounds).
            # Staging goes on ACT while it is idle (phase 1) and on DVE once
            # the EXP stream owns ACT. PSUM comes from the caller's pool:
            # phase-1 [128,512] tiles, or a wide [128,1024] attention tile.
            def q_proj_mms(ch, pack, pq, lo, hi):
                cs = slice(ch * 512, (ch + 1) * 512)
                for kb in range(lo, hi):
                    nc.tensor.matmul(
                        pq,
                        wq_sb[:, kb, pack * 128:(pack + 1) * 128],
                        xt_sb[:, kb, cs],
                        start=(kb == 0),
                        stop=(kb == NDB - 1),
                    )

            def q_rope_tail(ch, stage_act, pq, pack):
                cs = slice(ch * 512, (ch + 1) * 512)
                if True:
                    qs = stage.tile([128, 512], BF16, tag="qs")
                    if stage_act:
                        nc.scalar.copy(qs, pq)
                    else:
                        nc.vector.tensor_copy(qs, pq)
                    t1 = rtmp.tile([128, 512], BF16, tag="t1q")
                    t2 = rtmp.tile([128, 512], BF16, tag="t2q")
                    nc.vector.tensor_mul(t1, qs, cos2[:, cs])
                    for hh in range(2):
                        r = 64 * hh
                        nc.vector.tensor_mul(
                            t2[r:r + 32, :], qs[r + 32:r + 64, :],
                            sin2[r + 32:r + 64, cs],
                        )
                        nc.vector.tensor_mul(
                            t2[r + 32:r + 64, :], qs[r:r + 32, :],
                            sin2[r:r + 32, cs],
                        )
                    for hh in range(2):
                        h = pack * 2 + hh
                        r = 64 * hh
                        qcols = slice(h * N + ch * 512, h * N + (ch + 1) * 512)
                        if USE_FP8:
                            nc.vector.tensor_add(
                                q8[:, 0, qcols], t1[r:r + 32, :], t2[r:r + 32, :]
                            )
                            nc.vector.tensor_add(
                                q8[:, 1, qcols],
                                t1[r + 32:r + 64, :],
                                t2[r + 32:r + 64, :],
                            )
                        else:
                            nc.vector.tensor_add(
                                qt_sb[0:64, qcols],
                                t1[r:r + 64, :],
                                t2[r:r + 64, :],
                            )

            def q_proj_ch(ch, stage_act, psum_pool):
                for pack in range(2):
                    pq = psum_pool.tile([128, 512], F32, tag="pj",
                                        name=f"pq{ch}_{pack}")
                    q_proj_pack(ch, stage_act, pq, pack)

            # ---- attention + everything pools ----
            # PSUM: psS 3x[128,1024] (6 banks) + psO 2x[65,512] (2 banks).
            # KV projection, Q projection and the output projection all
            # borrow psS wide tiles, so the whole kernel is one stream.
            with (
                tc.tile_pool(name="ppool", bufs=6) as ppool,
                tc.tile_pool(name="unnp", bufs=10) as unnp,
                tc.tile_pool(name="den8p", bufs=5) as den8p,
                tc.tile_pool(name="rc8p", bufs=2) as rc8p,
                tc.tile_pool(name="bcp", bufs=2) as bcp,
                tc.tile_pool(name="psS", bufs=3, space="PSUM") as psS,
                tc.tile_pool(name="psO", bufs=2, space="PSUM") as psO,
                tc.tile_pool(name="ostage", bufs=6) as ostage,
            ):
                def kv_ch(ch):
                    kvt = psS.tile([128, 1024], F32, tag="s")
                    pkv = kvt[:, 0:512]
                    cs = slice(ch * 512, (ch + 1) * 512)
                    for kb in range(NDB):
                        nc.tensor.matmul(
                            pkv,
                            wkv_sb[:, kb, :],
                            xt_sb[:, kb, cs],
                            start=(kb == 0),
                            stop=(kb == NDB - 1),
                        )
                    ks = stage.tile([64, 512], BF16, tag="ks")
                    nc.scalar.copy(ks, pkv[0:64, :])
                    nc.scalar.copy(vt_sb[:, cs], pkv[64:128, :])
                    t1 = rtmp.tile([64, 512], BF16, tag="t1")
                    t2 = rtmp.tile([64, 512], BF16, tag="t2")
                    nc.vector.tensor_mul(t1, ks, cos2[0:64, cs])
                    nc.vector.tensor_mul(t2[0:32, :], ks[32:64, :], sin2[32:64, cs])
                    nc.vector.tensor_mul(t2[32:64, :], ks[0:32, :], sin2[0:32, cs])
                    if USE_FP8:
                        nc.vector.tensor_add(k8[:, 0, cs], t1[0:32, :], t2[0:32, :])
                        nc.vector.tensor_add(k8[:, 1, cs], t1[32:64, :], t2[32:64, :])
                    else:
                        nc.vector.tensor_add(kt_sb[0:64, cs], t1, t2)

                def v_trans(ch):
                    # V_aug blocks via PE transposes into a borrowed wide
                    # psS tile (fp32 path: transpose out dtype must match vt)
                    tr = psS.tile([128, 1024], F32, tag="s")
                    for i, t in enumerate(range(4 * ch, 4 * ch + 4)):
                        nc.tensor.transpose(
                            tr[:, i * 64:(i + 1) * 64],
                            vt_sb[:, t * 128:(t + 1) * 128],
                            id64,
                        )
                        nc.vector.tensor_copy(
                            vaug_sb[:, t, 0:64], tr[:, i * 64:(i + 1) * 64]
                        )
                        nc.vector.memset(vaug_sb[:, t, 64:65], 1.0)

                def q_proj_wide(ch, pack, stage_act=False):
                    pq = psS.tile([128, 1024], F32, tag="s")
                    q_proj_mms(ch, pack, pq[:, 0:512], 0, NDB)
                    q_rope_tail(ch, stage_act, pq[:, 0:512], pack)

                # prologue: projections feed the stream just-in-time; the
                # transposes trail one block so the PE covers staging latency
                nc.vector.memset(dpack, 1.0)
                kv_ch(0)
                q_proj_wide(0, 0, True)
                v_trans(0)
                kv_ch(1)
                q_proj_wide(0, 1, True)
                v_trans(1)
                kv_ch(2)
                q_proj_wide(1, 0, True)
                v_trans(2)
                kv_ch(3)
                q_proj_wide(1, 1, True)
                v_trans(3)

                # ---- software-pipelined attention stream ----
                # One global stream of (qh, h, kb) units. PV trails S by SKEW
                # units so the PE never queues behind the (pacing) EXP stream
                # at round boundaries. Normalization + output projection are
                # spliced in at PV-round completions.
                SKEW = 5
                units = [(qh, h, kb)
                         for qh in range(2) for h in range(HPC)
                         for kb in range(NKB)]
                NTILE = {0: [(0, 0), (0, 1), (0, 2), (0, 3),
                             (1, 0), (1, 1), (1, 2), (1, 3)],
                         1: [(0, 0), (0, 1), (0, 2), (0, 3),
                             (1, 0), (1, 1), (2, 0), (2, 1)]}

                dens = {}
                for qh in range(2):
                    ntiles = 1 + max(t for t, _ in NTILE[qh])
                    dens[qh] = [den8p.tile([97, 512], F32, tag="d8",
                                           name=f"den{qh}_{t}")
                                for t in range(ntiles)]
                    for t in range(ntiles):
                        nc.vector.memset(dens[qh][t], 1.0)
                unns = {0: [], 1: []}
                recs = {}
                po_of = {}
                pt_of = {}

                def emit_S(u):
                    qh, h, kb = units[u]
                    base = h * N + qh * 1024
                    ps = psS.tile([128, 1024], F32, tag="s")
                    for half in range(2):
                        qcols = slice(base + half * 512,
                                      base + (half + 1) * 512)
                        if USE_FP8:
                            nc.tensor.matmul(
                                ps[:, half * 512:(half + 1) * 512],
                                k8[:, :, kb * 128:(kb + 1) * 128],
                                q8[:, :, qcols],
                                start=True, stop=True, perf_mode=DROW,
                            )
                        else:
                            nc.tensor.matmul(
                                ps[:, half * 512:(half + 1) * 512],
                                kt_sb[:, kb * 128:(kb + 1) * 128],
                                qt_sb[:, qcols],
                                start=True, stop=True,
                            )
                    p_t = ppool.tile([128, 1024], BF16, tag="p")
                    nc.scalar.activation(p_t, ps, EXP, bias=0.0, scale=SCALE)
                    pt_of[u] = p_t

                def emit_PV(u):
                    qh, h, kb = units[u]
                    r = u // NKB
                    if kb == 0:
                        po_of[r] = [psO.tile([65, 512], F32, tag="o",
                                             name=f"po{r}_{i}")
                                    for i in range(2)]
                    po = po_of[r]
                    p_t = pt_of.pop(u)
                    for half in range(2):
                        nc.tensor.matmul(
                            po[half],
                            vaug_sb[:, kb, :],
                            p_t[:, half * 512:(half + 1) * 512],
                            start=(kb == 0),
                            stop=(kb == NKB - 1),
                            skip_group_check=True,
                        )
                    if kb == NKB - 1 and r != 7:
                        for half in range(2):
                            rr = h * 2 + half
                            unn = unnp.tile([64, 512], BF16, tag="unn",
                                            name=f"unn{qh}_{rr}")
                            nc.vector.tensor_copy(unn, po[half][0:64, :])
                            t, row = NTILE[qh][rr]
                            nc.vector.tensor_copy(
                                dens[qh][t][row * 32:row * 32 + 1, :],
                                po[half][64:65, :],
                            )
                            unns[qh].append(unn)

                def recip(qh, t):
                    rc = rc8p.tile([97, 512], F32, tag="rc8",
                                   name=f"rec{qh}_{t}")
                    nc.vector.reciprocal(rc, dens[qh][t])
                    recs[(qh, t)] = rc

                def norm_units(qh, rlist):
                    for r in rlist:
                        h, half = r // 2, r % 2
                        pair, row0 = h // 2, 64 * (h % 2)
                        t, row = NTILE[qh][r]
                        rr = rc8p.tile([1, 512], BF16, tag="rr")
                        nc.vector.tensor_copy(
                            rr, recs[(qh, t)][row * 32:row * 32 + 1, :])
                        bc = bcp.tile([64, 512], BF16, tag="bc")
                        nc.gpsimd.partition_broadcast(bc, rr)
                        ocols = slice(qh * 1024 + half * 512,
                                      qh * 1024 + (half + 1) * 512)
                        nc.vector.tensor_mul(
                            ot_sb[pair][row0:row0 + 64, ocols],
                            unns[qh][r],
                            bc,
                        )

                def out_proj(qh, qbs):
                    for qb in qbs:
                        pt = psS.tile([128, 1024], F32, tag="s")
                        for nch in range(2):
                            for pair in range(2):
                                nc.tensor.matmul(
                                    pt[:, nch * 512:(nch + 1) * 512],
                                    ot_sb[pair][:, qb * 128:(qb + 1) * 128],
                                    wo_sb[:, pair, nch * 512:(nch + 1) * 512],
                                    start=(pair == 0),
                                    stop=(pair == 1),
                                )
                        st = ostage.tile([128, 1024], F16, tag="st")
                        for nch in range(2):
                            nc.vector.tensor_copy(
                                st[:, nch * 512:(nch + 1) * 512],
                                pt[:, nch * 512:(nch + 1) * 512],
                            )
                        # two [64,1024] transfers (2KB descriptors), one per
                        # trigger queue, so the tail drains in parallel
                        for ph, e in ((0, nc.sync), (1, nc.gpsimd)):
                            e.dma_start(
                                out[qb * 128 + ph * 64:
                                    qb * 128 + (ph + 1) * 64, :],
                                st[ph * 64:(ph + 1) * 64, :],
                            )

                op_state = {}

                def op_a(qb):
                    pt = psS.tile([128, 1024], F32, tag="s")
                    op_state[qb] = pt
                    for nch in range(2):
                        nc.tensor.matmul(
                            pt[:, nch * 512:(nch + 1) * 512],
                            ot_sb[0][:, qb * 128:(qb + 1) * 128],
                            wo_sb[:, 0, nch * 512:(nch + 1) * 512],
                            start=True, stop=False,
                        )

                def op_b(qb, stage_act=False):
                    pt = op_state.pop(qb)
                    for nch in range(2):
                        nc.tensor.matmul(
                            pt[:, nch * 512:(nch + 1) * 512],
                            ot_sb[1][:, qb * 128:(qb + 1) * 128],
                            wo_sb[:, 1, nch * 512:(nch + 1) * 512],
                            start=False, stop=True,
                        )
                    st = ostage.tile([128, 1024], F16, tag="st")
                    for nch in range(2):
                        if stage_act:
                            nc.scalar.copy(
                                st[:, nch * 512:(nch + 1) * 512],
                                pt[:, nch * 512:(nch + 1) * 512],
                            )
                        else:
                            nc.vector.tensor_copy(
                                st[:, nch * 512:(nch + 1) * 512],
                                pt[:, nch * 512:(nch + 1) * 512],
                            )
                    for ph, e in ((0, nc.sync), (1, nc.gpsimd)):
                        e.dma_start(
                            out[qb * 128 + ph * 64:
                                qb * 128 + (ph + 1) * 64, :],
                            st[ph * 64:(ph + 1) * 64, :],
                        )

                # qp splices split into 4-matmul halves so the EXP
                # backlog fully hides each insertion
                qp_state = {}

                def qp_h1(ch, pack):
                    pq = psS.tile([128, 1024], F32, tag="s")
                    qp_state[(ch, pack)] = pq
                    q_proj_mms(ch, pack, pq[:, 0:512], 0, 4)

                def qp_h2(ch, pack):
                    pq = qp_state.pop((ch, pack))
                    q_proj_mms(ch, pack, pq[:, 0:512], 4, NDB)
                    q_rope_tail(ch, False, pq[:, 0:512], pack)

                def tail_norm_out():
                    # round 7 (qh1, h3): pack both halves' denominators into
                    # one tile (rows 0/32) so ONE reciprocal covers them; the
                    # first chunks' pair0 matmuls run on the PE while the
                    # reciprocal chain occupies the DVE; staging alternates
                    # DVE/ACT (ACT is idle in the tail).
                    po = po_of[7]
                    nc.vector.tensor_copy(dpack[0:1, :], po[0][64:65, :])
                    nc.vector.tensor_copy(dpack[32:33, :], po[1][64:65, :])
                    rcp = rc8p.tile([33, 512], F32, tag="rcp")
                    nc.vector.reciprocal(rcp, dpack)
                    for qb in (8, 9, 10):
                        op_a(qb)
                    for half in range(2):
                        rbT = rc8p.tile([1, 512], BF16, tag="rbt")
                        nc.vector.tensor_copy(
                            rbT, rcp[32 * half:32 * half + 1, :])
                        bcT = bcp.tile([64, 512], BF16, tag="bct")
                        nc.gpsimd.partition_broadcast(bcT, rbT)
                        oc = slice(1024 + half * 512,
                                   1024 + (half + 1) * 512)
                        nc.vector.tensor_mul(
                            ot_sb[1][64:128, oc],
                            po[half][0:64, :],
                            bcT,
                        )
                        if half == 0:
                            for qb in (8, 9, 10):
                                op_b(qb, stage_act=True)
                            op_a(11)
                            op_b(11, stage_act=True)
                        else:
                            for qb in (12, 13, 14, 15):
                                op_a(qb)
                                op_b(qb, stage_act=True)

                # Splice points: small chunks of projection / normalize /
                # output work dropped between stream units so the PE never
                # inserts more work than the EXP backlog can hide.
                splice = {
                    13: [lambda: qp_h1(2, 0)],
                    15: [lambda: qp_h2(2, 0)],
                    21: [lambda: qp_h1(2, 1)],
                    23: [lambda: qp_h2(2, 1)],
                    33: [lambda: recip(0, 0)],
                    34: [lambda: norm_units(0, [0])],
                    36: [lambda: norm_units(0, [1])],
                    37: [lambda: qp_h1(3, 0)],
                    38: [lambda: norm_units(0, [2])],
                    39: [lambda: qp_h2(3, 0)],
                    40: [lambda: norm_units(0, [3])],
                    45: [lambda: qp_h1(3, 1)],
                    47: [lambda: qp_h2(3, 1)],
                    65: [lambda: recip(0, 1)],
                    66: [lambda: norm_units(0, [4])],
                    68: [lambda: norm_units(0, [5])],
                    70: [lambda: norm_units(0, [6])],
                    72: [lambda: norm_units(0, [7])],
                    74: [lambda: op_a(0)],
                    76: [lambda: op_b(0)],
                    78: [lambda: op_a(1)],
                    80: [lambda: op_b(1)],
                    82: [lambda: op_a(2)],
                    84: [lambda: op_b(2)],
                    86: [lambda: op_a(3)],
                    88: [lambda: op_b(3)],
                    90: [lambda: op_a(4)],
                    92: [lambda: op_b(4)],
                    94: [lambda: op_a(5)],
                    96: [lambda: op_b(5)],
                    97: [lambda: recip(1, 0)],
                    99: [lambda: norm_units(1, [0])],
                    100: [lambda: op_a(6)],
                    101: [lambda: norm_units(1, [1])],
                    102: [lambda: op_b(6)],
                    103: [lambda: norm_units(1, [2])],
                    104: [lambda: op_a(7)],
                    105: [lambda: norm_units(1, [3])],
                    106: [lambda: op_b(7)],
                    113: [lambda: recip(1, 1)],
                    115: [lambda: norm_units(1, [4])],
                    117: [lambda: norm_units(1, [5])],
                    127: [tail_norm_out],
                }

                NU = len(units)
                for u in range(NU + SKEW):
                    if u < NU:
                        emit_S(u)
                    v = u - SKEW
                    if v >= 0:
                        emit_PV(v)
                        for fn in splice.get(v, []):
                            fn()

    es.close()


def _rope_tables():
    inv_freq = 1.0 / (10000.0 ** (np.arange(0, D, 2, dtype=np.float64) / D))
    freqs = np.outer(np.arange(N, dtype=np.float64), inv_freq)  # [N, 32]
    cos_h = np.cos(freqs).astype(np.float32).T                  # [32, N]
    sin_h = np.sin(freqs).astype(np.float32).T                  # [32, N]
    cost = np.concatenate([cos_h, cos_h], 0)                    # [64, N]
    # sign layout [+sin; -sin]: the crossing multiplies index this table at
    # the SOURCE partitions (walrus requires equal base partitions for
    # SBUF+SBUF tensor_tensor inputs), so row r holds the sign of the row it
    # multiplies INTO the other half.
    sincat = np.concatenate([sin_h, -sin_h], 0)                 # [64, N]
    return np.ascontiguousarray(cost), np.ascontiguousarray(sincat)


@functools.lru_cache(maxsize=1)
def _program():
    nc = bacc.Bacc(
        "TRN2", target_bir_lowering=False, debug=False, enable_asserts=False
    )
    io = {
        "xt": nc.dram_tensor("xt", [NDB, 128, N], BF16, kind="ExternalInput").ap(),
        "wq": nc.dram_tensor("wq", [NDB, 128, 256], BF16, kind="ExternalInput").ap(),
        "wkv": nc.dram_tensor("wkv", [NDB, 128, 128], BF16, kind="ExternalInput").ap(),
        "wo": nc.dram_tensor("wo", [2, 128, DIM], BF16, kind="ExternalInput").ap(),
        "cost": nc.dram_tensor("cost", [64, N], BF16, kind="ExternalInput").ap(),
        "sincat": nc.dram_tensor("sincat", [64, N], BF16, kind="ExternalInput").ap(),
        "out": nc.dram_tensor("out", [N, DIM], F16, kind="ExternalOutput").ap(),
    }
    with tile.TileContext(nc) as tc:
        _build(nc, tc, io)
    nc.compile()
    return nc


def make_in_maps(x, Wq, Wkv, Wo):
    import ml_dtypes

    bf16 = ml_dtypes.bfloat16
    cost, sincat = _rope_tables()
    in_maps = []
    for c in range(8):
        b, j = c // 4, c % 4
        xt = np.ascontiguousarray(x[b].T).reshape(NDB, 128, N)
        wq_c = np.ascontiguousarray(Wq[:, 256 * j:256 * (j + 1)]).reshape(
            NDB, 128, 256
        )
        wkv_c = np.ascontiguousarray(
            np.concatenate(
                [Wkv[:, 64 * j:64 * (j + 1)],
                 Wkv[:, 256 + 64 * j:256 + 64 * (j + 1)]],
                axis=1,
            )
        ).reshape(NDB, 128, 128)
        wo_c = np.ascontiguousarray(Wo[256 * j:256 * (j + 1), :]).reshape(
            2, 128, DIM
        )
        in_maps.append(
            {
                "xt": xt.astype(bf16),
                "wq": wq_c.astype(bf16),
                "wkv": wkv_c.astype(bf16),
                "wo": wo_c.astype(bf16),
                "cost": cost.astype(bf16),
                "sincat": sincat.astype(bf16),
            }
        )
    return in_maps


def _install_ntff_hook():
    """Register the axon NTFF profiling hook that this image's antenv lacks."""
    import types

    if "antenv.axon_hooks" in sys.modules:
        return
    try:
        sys.path.append("/root/.axon_site")
        from trn_agent_boot.trn_boot import _ntff_profile_via_ctypes

        hook = _ntff_profile_via_ctypes("/opt/axon/libaxon_pjrt.so")
    except Exception:
        hook = None
    finally:
        try:
            sys.path.remove("/root/.axon_site")
        except ValueError:
            pass
    mod = types.ModuleType("antenv.axon_hooks")
    mod.get_axon_ntff_profile_hook = lambda: hook
    mod.set_axon_ntff_profile_hook = lambda h: None
    sys.modules["antenv.axon_hooks"] = mod
    # artifact upload needs bucket credentials this container lacks
    import concourse.bass_utils as bu

    bu.upload_artifacts = lambda tmpdir: "local://" + str(tmpdir)


def kernel(x, Wq, Wkv, Wo, bo):
    from concourse.bass_utils import run_bass_kernel_spmd

    _install_ntff_hook()
    nc = _program()
    in_maps = make_in_maps(x, Wq, Wkv, Wo)
    trace = bool(os.environ.get("KERNEL_TRACE"))
    res = run_bass_kernel_spmd(
        nc, in_maps, list(range(8)), trace=trace
    )
    LAST_RESULTS["res"] = res
    full = np.zeros((B, N, DIM), np.float32)
    for c in range(8):
        full[c // 4] += res.results[c]["out"].astype(np.float32)
    full += bo.astype(np.float32)
    return full



# revision 54
# speedup vs baseline: 1.1977x; 1.0123x over previous
"""GQA attention (16 Q heads / 4 KV heads, RoPE, n=2048, d=64) on 8 trn2 cores.

Sharding: core c = (batch b=c//4, kv-group j=c%4). Each core owns 4 query
heads sharing one KV head, computes its partial output projection
(O_heads @ Wo_rows), and the host sums the 4 partials per batch.

Pipeline per core (bf16 matmuls, fp32 PSUM accumulation):
  x DMA'd ch-major so the KV projection starts early; projections run as a
  dense PE prologue, then a software-pipelined attention stream of
  (q-chunk, head, key-block) units paced by the EXP activations on ACT
  (~1.11us per [128,1024] tile; the stream runs ~97% exp-dense).
  S^T = K^T Q bf16 (128-partition contraction, zero-padded rows -- 64-row
    matmuls are much slower on HW); P = exp on ACT; O^T+denom from matmul
    with V_aug (ones column -> denominator).
  Normalize/output-projection work is spliced between stream units in
  small pieces (single norm units, pair-split out_proj halves two units
  apart) so no splice exceeds the exp backlog.
  Tail (last round, queries 1024:2048): both denominator halves packed
  into one tile for a single reciprocal; the first output chunks' pair0
  matmuls run on the PE under the reciprocal; PSUM->SBUF staging
  alternates DVE/ACT (ACT is idle in the tail); fp16 output on the
  sync+gpsimd trigger queues (never ACT mid-stream -- a waiting DMA
  trigger there head-of-line blocks the exps).
"""

import os
import sys
import functools

import numpy as np

sys.path.insert(0, "/opt/trn_rl_repo")

import concourse.bass as bass  # noqa: E402
import concourse.bacc as bacc  # noqa: E402
import concourse.tile as tile  # noqa: E402
import concourse.mybir as mybir  # noqa: E402
from concourse.masks import make_identity  # noqa: E402

F32 = mybir.dt.float32
BF16 = mybir.dt.bfloat16
F16 = mybir.dt.float16
FP8 = mybir.dt.float8e4
EXP = mybir.ActivationFunctionType.Exp
DROW = mybir.MatmulPerfMode.DoubleRow

B, N, DIM = 2, 2048, 1024
HEADS, KVH, D = 16, 4, 64
HPC = HEADS // KVH          # q heads per core = 4
SCALE = D ** -0.5           # 1/8
QTOT = HPC * N              # 8192 concatenated query columns
NKB = N // 128              # 16 key blocks
NDB = DIM // 128            # 8 contraction blocks for projections

USE_FP8 = os.environ.get("KERNEL_FP8") == "1"

LAST_RESULTS = {}           # test.py introspection


def _build(nc, tc, io):
    from contextlib import ExitStack

    xt, wq, wkv, wo = io["xt"], io["wq"], io["wkv"], io["wo"]
    cost, sincat, out = io["cost"], io["sincat"], io["out"]

    es = ExitStack()
    consts = es.enter_context(tc.tile_pool(name="consts", bufs=1))
    acts = es.enter_context(tc.tile_pool(name="acts", bufs=1))

    wq_sb = consts.tile([128, NDB, 256], BF16, tag="wq")
    wkv_sb = consts.tile([128, NDB, 128], BF16, tag="wkv")
    wo_sb = consts.tile([128, 2, DIM], BF16, tag="wo")
    cos2 = consts.tile([128, N], BF16, tag="cos2")
    sin2 = consts.tile([128, N], BF16, tag="sin2")
    id64 = consts.tile([64, 64], F32, tag="id")

    vt_sb = acts.tile([64, N], F32, tag="vt")
    vaug_sb = acts.tile([128, NKB, 65], BF16, tag="vaug")
    ot_sb = [
        acts.tile([128, N], BF16, tag=f"ot{i}", name=f"ot{i}") for i in range(2)
    ]
    if USE_FP8:
        k8 = acts.tile([32, 2, N], FP8, tag="k8")
        q8 = acts.tile([32, 2, QTOT], FP8, tag="q8")
    else:
        kt_sb = acts.tile([128, N], BF16, tag="kt")
        qt_sb = acts.tile([128, QTOT], BF16, tag="qt")

    ones64 = consts.tile([1, 64], BF16, tag="ones64")
    dpack = acts.tile([33, 512], F32, tag="dpk")

    with tc.tile_pool(name="xtp", bufs=1) as xt_pool:
        xt_sb = xt_pool.tile([128, NDB, N], BF16, tag="xt")

        # DMA issue order: KV weights + ch0 tables + ch0 x first. Early
        # triggers alternate between the two HWDGE queues (SP + Act) so the
        # ~0.6us-per-trigger dispatch doesn't serialize the lead-in.
        # x (the bulk) goes through gpsimd SWDGE triggers: ~25ns dispatch
        # each vs ~600ns on the HWDGE queues, so all 32 transfers are in
        # flight almost immediately. Weights/tables ride the two HWDGE
        # queues in priority order.
        eng = [nc.sync, nc.scalar]
        # identity first: it is tiny and the V transposes need it early,
        # while the big kt/qt memsets wait until after the x triggers.
        make_identity(nc, id64)
        # all of x through gpsimd SWDGE triggers (own queue, in flight
        # early); weights/tables split across the two HWDGE queues in
        # priority order.
        for ch in range(4):
            for kb in range(NDB):
                nc.gpsimd.dma_start(
                    xt_sb[:, kb, ch * 512:(ch + 1) * 512],
                    xt[kb, :, ch * 512:(ch + 1) * 512],
                )
        nc.vector.memset(ones64, 1.0)
        if not USE_FP8:
            nc.gpsimd.memset(kt_sb[64:128, :], 0.0)
            nc.gpsimd.memset(qt_sb[64:128, :], 0.0)
        for g in range(4):
            eng[g % 2].dma_start(
                wkv_sb[:, 2 * g:2 * g + 2, :],
                wkv[2 * g:2 * g + 2].transpose([1, 0, 2]),
            )
        eng[0].dma_start(cos2[0:64, 0:512], cost[:, 0:512])
        eng[1].dma_start(sin2[0:64, 0:512], sincat[:, 0:512])
        eng[0].dma_start(cos2[0:64, 512:1024], cost[:, 512:1024])
        eng[1].dma_start(sin2[0:64, 512:1024], sincat[:, 512:1024])
        for kb in range(NDB):
            eng[kb % 2].dma_start(wq_sb[:, kb, :], wq[kb])
        eng[0].dma_start(cos2[0:64, 1024:N], cost[:, 1024:N])
        eng[1].dma_start(sin2[0:64, 1024:N], sincat[:, 1024:N])
        eng[0].dma_start(cos2[64:128, 0:1024], cost[:, 0:1024])
        eng[1].dma_start(sin2[64:128, 0:1024], sincat[:, 0:1024])
        eng[0].dma_start(cos2[64:128, 1024:N], cost[:, 1024:N])
        eng[1].dma_start(sin2[64:128, 1024:N], sincat[:, 1024:N])
        for pair in range(2):
            for half in range(2):
                eng[half].dma_start(
                    wo_sb[:, pair, half * 512:(half + 1) * 512],
                    wo[pair][:, half * 512:(half + 1) * 512],
                )

        with (
            tc.tile_pool(name="stage", bufs=4) as stage,
            tc.tile_pool(name="rtmp", bufs=4) as rtmp,
        ):
            # ---- Q projection + rope (pipelined into the attention rounds).
            # Staging goes on ACT while it is idle (phase 1) and on DVE once
            # the EXP stream owns ACT. PSUM comes from the caller's pool:
            # phase-1 [128,512] tiles, or a wide [128,1024] attention tile.
            def q_proj_mms(ch, pack, pq, lo, hi):
                cs = slice(ch * 512, (ch + 1) * 512)
                for kb in range(lo, hi):
                    nc.tensor.matmul(
                        pq,
                        wq_sb[:, kb, pack * 128:(pack + 1) * 128],
                        xt_sb[:, kb, cs],
                        start=(kb == 0),
                        stop=(kb == NDB - 1),
                    )

            def q_rope_tail(ch, stage_act, pq, pack):
                cs = slice(ch * 512, (ch + 1) * 512)
                if True:
                    qs = stage.tile([128, 512], BF16, tag="qs")
                    if stage_act:
                        nc.scalar.copy(qs, pq)
                    else:
                        nc.vector.tensor_copy(qs, pq)
                    t1 = rtmp.tile([128, 512], BF16, tag="t1q")
                    t2 = rtmp.tile([128, 512], BF16, tag="t2q")
                    nc.vector.tensor_mul(t1, qs, cos2[:, cs])
                    for hh in range(2):
                        r = 64 * hh
                        nc.vector.tensor_mul(
                            t2[r:r + 32, :], qs[r + 32:r + 64, :],
                            sin2[r + 32:r + 64, cs],
                        )
                        nc.vector.tensor_mul(
                            t2[r + 32:r + 64, :], qs[r:r + 32, :],
                            sin2[r:r + 32, cs],
                        )
                    for hh in range(2):
                        h = pack * 2 + hh
                        r = 64 * hh
                        qcols = slice(h * N + ch * 512, h * N + (ch + 1) * 512)
                        if USE_FP8:
                            nc.vector.tensor_add(
                                q8[:, 0, qcols], t1[r:r + 32, :], t2[r:r + 32, :]
                            )
                            nc.vector.tensor_add(
                                q8[:, 1, qcols],
                                t1[r + 32:r + 64, :],
                                t2[r + 32:r + 64, :],
                            )
                        else:
                            nc.vector.tensor_add(
                                qt_sb[0:64, qcols],
                                t1[r:r + 64, :],
                                t2[r:r + 64, :],
                            )

            def q_proj_ch(ch, stage_act, psum_pool):
                for pack in range(2):
                    pq = psum_pool.tile([128, 512], F32, tag="pj",
                                        name=f"pq{ch}_{pack}")
                    q_proj_pack(ch, stage_act, pq, pack)

            # ---- attention + everything pools ----
            # PSUM: psS 3x[128,1024] (6 banks) + psO 2x[65,512] (2 banks).
            # KV projection, Q projection and the output projection all
            # borrow psS wide tiles, so the whole kernel is one stream.
            with (
                tc.tile_pool(name="ppool", bufs=7) as ppool,
                tc.tile_pool(name="unnp", bufs=10) as unnp,
                tc.tile_pool(name="den8p", bufs=5) as den8p,
                tc.tile_pool(name="rc8p", bufs=3) as rc8p,
                tc.tile_pool(name="bcp", bufs=3) as bcp,
                tc.tile_pool(name="psS", bufs=3, space="PSUM") as psS,
                tc.tile_pool(name="psO", bufs=2, space="PSUM") as psO,
                tc.tile_pool(name="ostage", bufs=4) as ostage,
            ):
                def kv_ch(ch):
                    kvt = psS.tile([128, 1024], F32, tag="s")
                    pkv = kvt[:, 0:512]
                    cs = slice(ch * 512, (ch + 1) * 512)
                    for kb in range(NDB):
                        nc.tensor.matmul(
                            pkv,
                            wkv_sb[:, kb, :],
                            xt_sb[:, kb, cs],
                            start=(kb == 0),
                            stop=(kb == NDB - 1),
                        )
                    ks = stage.tile([64, 512], BF16, tag="ks")
                    nc.scalar.copy(ks, pkv[0:64, :])
                    nc.scalar.copy(vt_sb[:, cs], pkv[64:128, :])
                    t1 = rtmp.tile([64, 512], BF16, tag="t1")
                    t2 = rtmp.tile([64, 512], BF16, tag="t2")
                    nc.vector.tensor_mul(t1, ks, cos2[0:64, cs])
                    nc.vector.tensor_mul(t2[0:32, :], ks[32:64, :], sin2[32:64, cs])
                    nc.vector.tensor_mul(t2[32:64, :], ks[0:32, :], sin2[0:32, cs])
                    if USE_FP8:
                        nc.vector.tensor_add(k8[:, 0, cs], t1[0:32, :], t2[0:32, :])
                        nc.vector.tensor_add(k8[:, 1, cs], t1[32:64, :], t2[32:64, :])
                    else:
                        nc.vector.tensor_add(kt_sb[0:64, cs], t1, t2)

                def v_trans(ch):
                    # V_aug blocks via PE transposes into a borrowed wide
                    # psS tile (fp32 path: transpose out dtype must match vt)
                    tr = psS.tile([128, 1024], F32, tag="s")
                    for i, t in enumerate(range(4 * ch, 4 * ch + 4)):
                        nc.tensor.transpose(
                            tr[:, i * 64:(i + 1) * 64],
                            vt_sb[:, t * 128:(t + 1) * 128],
                            id64,
                        )
                        nc.vector.tensor_copy(
                            vaug_sb[:, t, 0:64], tr[:, i * 64:(i + 1) * 64]
                        )
                        nc.vector.memset(vaug_sb[:, t, 64:65], 1.0)

                def q_proj_wide(ch, pack, stage_act=False):
                    pq = psS.tile([128, 1024], F32, tag="s")
                    q_proj_mms(ch, pack, pq[:, 0:512], 0, NDB)
                    q_rope_tail(ch, stage_act, pq[:, 0:512], pack)

                # prologue: projections feed the stream just-in-time; the
                # transposes trail one block so the PE covers staging latency
                nc.vector.memset(dpack, 1.0)
                kv_ch(0)
                q_proj_wide(0, 0, True)
                v_trans(0)
                kv_ch(1)
                q_proj_wide(0, 1, True)
                v_trans(1)
                kv_ch(2)
                q_proj_wide(1, 0, True)
                v_trans(2)
                kv_ch(3)
                q_proj_wide(1, 1, True)
                v_trans(3)

                # ---- software-pipelined attention stream ----
                # One global stream of (qh, h, kb) units. PV trails S by SKEW
                # units so the PE never queues behind the (pacing) EXP stream
                # at round boundaries. Normalization + output projection are
                # spliced in at PV-round completions.
                SKEW = 5
                units = [(qh, h, kb)
                         for qh in range(2) for h in range(HPC)
                         for kb in range(NKB)]
                NTILE = {0: [(0, 0), (0, 1), (0, 2), (0, 3),
                             (1, 0), (1, 1), (1, 2), (1, 3)],
                         1: [(0, 0), (0, 1), (0, 2), (0, 3),
                             (1, 0), (1, 1), (2, 0), (2, 1)]}

                dens = {}
                for qh in range(2):
                    ntiles = 1 + max(t for t, _ in NTILE[qh])
                    dens[qh] = [den8p.tile([97, 512], F32, tag="d8",
                                           name=f"den{qh}_{t}")
                                for t in range(ntiles)]
                    for t in range(ntiles):
                        nc.vector.memset(dens[qh][t], 1.0)
                unns = {0: [], 1: []}
                recs = {}
                po_of = {}
                pt_of = {}

                def emit_S(u):
                    qh, h, kb = units[u]
                    base = h * N + qh * 1024
                    ps = psS.tile([128, 1024], F32, tag="s")
                    for half in range(2):
                        qcols = slice(base + half * 512,
                                      base + (half + 1) * 512)
                        if USE_FP8:
                            nc.tensor.matmul(
                                ps[:, half * 512:(half + 1) * 512],
                                k8[:, :, kb * 128:(kb + 1) * 128],
                                q8[:, :, qcols],
                                start=True, stop=True, perf_mode=DROW,
                            )
                        else:
                            nc.tensor.matmul(
                                ps[:, half * 512:(half + 1) * 512],
                                kt_sb[:, kb * 128:(kb + 1) * 128],
                                qt_sb[:, qcols],
                                start=True, stop=True,
                            )
                    p_t = ppool.tile([128, 1024], BF16, tag="p")
                    nc.scalar.activation(p_t, ps, EXP, bias=0.0, scale=SCALE)
                    pt_of[u] = p_t

                def emit_PV(u):
                    qh, h, kb = units[u]
                    r = u // NKB
                    if kb == 0:
                        po_of[r] = [psO.tile([65, 512], F32, tag="o",
                                             name=f"po{r}_{i}")
                                    for i in range(2)]
                    po = po_of[r]
                    p_t = pt_of.pop(u)
                    for half in range(2):
                        nc.tensor.matmul(
                            po[half],
                            vaug_sb[:, kb, :],
                            p_t[:, half * 512:(half + 1) * 512],
                            start=(kb == 0),
                            stop=(kb == NKB - 1),
                            skip_group_check=True,
                        )
                    if kb == NKB - 1 and r != 7:
                        for half in range(2):
                            rr = h * 2 + half
                            unn = unnp.tile([64, 512], BF16, tag="unn",
                                            name=f"unn{qh}_{rr}")
                            nc.vector.tensor_copy(unn, po[half][0:64, :])
                            t, row = NTILE[qh][rr]
                            nc.vector.tensor_copy(
                                dens[qh][t][row * 32:row * 32 + 1, :],
                                po[half][64:65, :],
                            )
                            unns[qh].append(unn)

                def recip(qh, t):
                    rc = rc8p.tile([97, 512], F32, tag="rc8",
                                   name=f"rec{qh}_{t}")
                    nc.vector.reciprocal(rc, dens[qh][t])
                    recs[(qh, t)] = rc

                def norm_units(qh, rlist):
                    for r in rlist:
                        h, half = r // 2, r % 2
                        pair, row0 = h // 2, 64 * (h % 2)
                        t, row = NTILE[qh][r]
                        rr = rc8p.tile([1, 512], BF16, tag="rr")
                        nc.vector.tensor_copy(
                            rr, recs[(qh, t)][row * 32:row * 32 + 1, :])
                        bc = bcp.tile([64, 512], BF16, tag="bc")
                        nc.gpsimd.partition_broadcast(bc, rr)
                        ocols = slice(qh * 1024 + half * 512,
                                      qh * 1024 + (half + 1) * 512)
                        nc.vector.tensor_mul(
                            ot_sb[pair][row0:row0 + 64, ocols],
                            unns[qh][r],
                            bc,
                        )

                def out_proj(qh, qbs):
                    for qb in qbs:
                        pt = psS.tile([128, 1024], F32, tag="s")
                        for nch in range(2):
                            for pair in range(2):
                                nc.tensor.matmul(
                                    pt[:, nch * 512:(nch + 1) * 512],
                                    ot_sb[pair][:, qb * 128:(qb + 1) * 128],
                                    wo_sb[:, pair, nch * 512:(nch + 1) * 512],
                                    start=(pair == 0),
                                    stop=(pair == 1),
                                )
                        st = ostage.tile([128, 1024], F16, tag="st")
                        for nch in range(2):
                            nc.vector.tensor_copy(
                                st[:, nch * 512:(nch + 1) * 512],
                                pt[:, nch * 512:(nch + 1) * 512],
                            )
                        # two [64,1024] transfers (2KB descriptors), one per
                        # trigger queue, so the tail drains in parallel
                        for ph, e in ((0, nc.sync), (1, nc.gpsimd)):
                            e.dma_start(
                                out[qb * 128 + ph * 64:
                                    qb * 128 + (ph + 1) * 64, :],
                                st[ph * 64:(ph + 1) * 64, :],
                            )

                op_state = {}

                def op_a(qb):
                    pt = psS.tile([128, 1024], F32, tag="s")
                    op_state[qb] = pt
                    for nch in range(2):
                        nc.tensor.matmul(
                            pt[:, nch * 512:(nch + 1) * 512],
                            ot_sb[0][:, qb * 128:(qb + 1) * 128],
                            wo_sb[:, 0, nch * 512:(nch + 1) * 512],
                            start=True, stop=False,
                        )

                def op_b(qb, stage_act=False):
                    pt = op_state.pop(qb)
                    for nch in range(2):
                        nc.tensor.matmul(
                            pt[:, nch * 512:(nch + 1) * 512],
                            ot_sb[1][:, qb * 128:(qb + 1) * 128],
                            wo_sb[:, 1, nch * 512:(nch + 1) * 512],
                            start=False, stop=True,
                        )
                    st = ostage.tile([128, 1024], F16, tag="st")
                    for nch in range(2):
                        if stage_act:
                            nc.scalar.copy(
                                st[:, nch * 512:(nch + 1) * 512],
                                pt[:, nch * 512:(nch + 1) * 512],
                            )
                        else:
                            nc.vector.tensor_copy(
                                st[:, nch * 512:(nch + 1) * 512],
                                pt[:, nch * 512:(nch + 1) * 512],
                            )
                    for ph, e in ((0, nc.sync), (1, nc.gpsimd)):
                        e.dma_start(
                            out[qb * 128 + ph * 64:
                                qb * 128 + (ph + 1) * 64, :],
                            st[ph * 64:(ph + 1) * 64, :],
                        )

                # qp splices split into 4-matmul halves so the EXP
                # backlog fully hides each insertion
                qp_state = {}

                def qp_h1(ch, pack):
                    pq = psS.tile([128, 1024], F32, tag="s")
                    qp_state[(ch, pack)] = pq
                    q_proj_mms(ch, pack, pq[:, 0:512], 0, 4)

                def qp_h2(ch, pack):
                    pq = qp_state.pop((ch, pack))
                    q_proj_mms(ch, pack, pq[:, 0:512], 4, NDB)
                    q_rope_tail(ch, False, pq[:, 0:512], pack)

                def tail_norm_out():
                    # round 7 (qh1, h3): pack both halves' denominators into
                    # one tile (rows 0/32) so ONE reciprocal covers them; the
                    # first chunks' pair0 matmuls run on the PE while the
                    # reciprocal chain occupies the DVE; staging alternates
                    # DVE/ACT (ACT is idle in the tail).
                    po = po_of[7]
                    nc.vector.tensor_copy(dpack[0:1, :], po[0][64:65, :])
                    nc.vector.tensor_copy(dpack[32:33, :], po[1][64:65, :])
                    rcp = rc8p.tile([33, 512], F32, tag="rcp")
                    nc.vector.reciprocal(rcp, dpack)
                    for qb in (8, 9, 10):
                        op_a(qb)
                    for half in range(2):
                        rbT = rc8p.tile([1, 512], BF16, tag="rbt")
                        nc.vector.tensor_copy(
                            rbT, rcp[32 * half:32 * half + 1, :])
                        bcT = bcp.tile([64, 512], BF16, tag="bct")
                        nc.gpsimd.partition_broadcast(bcT, rbT)
                        oc = slice(1024 + half * 512,
                                   1024 + (half + 1) * 512)
                        nc.vector.tensor_mul(
                            ot_sb[1][64:128, oc],
                            po[half][0:64, :],
                            bcT,
                        )
                        if half == 0:
                            for i, qb in enumerate((8, 9, 10)):
                                op_b(qb, stage_act=(i % 2 == 1))
                            op_a(11)
                            op_b(11)
                        else:
                            for i, qb in enumerate((12, 13, 14, 15)):
                                op_a(qb)
                                op_b(qb, stage_act=(i % 2 == 1))

                # Splice points: small chunks of projection / normalize /
                # output work dropped between stream units so the PE never
                # inserts more work than the EXP backlog can hide.
                splice = {
                    13: [lambda: qp_h1(2, 0)],
                    15: [lambda: qp_h2(2, 0)],
                    21: [lambda: qp_h1(2, 1)],
                    23: [lambda: qp_h2(2, 1)],
                    33: [lambda: recip(0, 0)],
                    34: [lambda: norm_units(0, [0])],
                    36: [lambda: norm_units(0, [1])],
                    37: [lambda: qp_h1(3, 0)],
                    38: [lambda: norm_units(0, [2])],
                    39: [lambda: qp_h2(3, 0)],
                    40: [lambda: norm_units(0, [3])],
                    45: [lambda: qp_h1(3, 1)],
                    47: [lambda: qp_h2(3, 1)],
                    65: [lambda: recip(0, 1)],
                    66: [lambda: norm_units(0, [4])],
                    68: [lambda: norm_units(0, [5])],
                    70: [lambda: norm_units(0, [6])],
                    72: [lambda: norm_units(0, [7])],
                    74: [lambda: op_a(0)],
                    76: [lambda: op_b(0)],
                    78: [lambda: op_a(1)],
                    80: [lambda: op_b(1)],
                    82: [lambda: op_a(2)],
                    84: [lambda: op_b(2)],
                    86: [lambda: op_a(3)],
                    88: [lambda: op_b(3)],
                    90: [lambda: op_a(4)],
                    92: [lambda: op_b(4)],
                    94: [lambda: op_a(5)],
                    96: [lambda: op_b(5)],
                    97: [lambda: recip(1, 0)],
                    99: [lambda: norm_units(1, [0])],
                    100: [lambda: op_a(6)],
                    101: [lambda: norm_units(1, [1])],
                    102: [lambda: op_b(6)],
                    103: [lambda: norm_units(1, [2])],
                    104: [lambda: op_a(7)],
                    105: [lambda: norm_units(1, [3])],
                    106: [lambda: op_b(7)],
                    113: [lambda: recip(1, 1)],
                    115: [lambda: norm_units(1, [4])],
                    117: [lambda: norm_units(1, [5])],
                    127: [tail_norm_out],
                }

                NU = len(units)
                for u in range(NU + SKEW):
                    if u < NU:
                        emit_S(u)
                    v = u - SKEW
                    if v >= 0:
                        emit_PV(v)
                        for fn in splice.get(v, []):
                            fn()

    es.close()


def _rope_tables():
    inv_freq = 1.0 / (10000.0 ** (np.arange(0, D, 2, dtype=np.float64) / D))
    freqs = np.outer(np.arange(N, dtype=np.float64), inv_freq)  # [N, 32]
    cos_h = np.cos(freqs).astype(np.float32).T                  # [32, N]
    sin_h = np.sin(freqs).astype(np.float32).T                  # [32, N]
    cost = np.concatenate([cos_h, cos_h], 0)                    # [64, N]
    # sign layout [+sin; -sin]: the crossing multiplies index this table at
    # the SOURCE partitions (walrus requires equal base partitions for
    # SBUF+SBUF tensor_tensor inputs), so row r holds the sign of the row it
    # multiplies INTO the other half.
    sincat = np.concatenate([sin_h, -sin_h], 0)                 # [64, N]
    return np.ascontiguousarray(cost), np.ascontiguousarray(sincat)


@functools.lru_cache(maxsize=1)
def _program():
    nc = bacc.Bacc(
        "TRN2", target_bir_lowering=False, debug=False, enable_asserts=False
    )
    io = {
        "xt": nc.dram_tensor("xt", [NDB, 128, N], BF16, kind="ExternalInput").ap(),
        "wq": nc.dram_tensor("wq", [NDB, 128, 256], BF16, kind="ExternalInput").ap(),
        "wkv": nc.dram_tensor("wkv", [NDB, 128, 128], BF16, kind="ExternalInput").ap(),
        "wo": nc.dram_tensor("wo", [2, 128, DIM], BF16, kind="ExternalInput").ap(),
        "cost": nc.dram_tensor("cost", [64, N], BF16, kind="ExternalInput").ap(),
        "sincat": nc.dram_tensor("sincat", [64, N], BF16, kind="ExternalInput").ap(),
        "out": nc.dram_tensor("out", [N, DIM], F16, kind="ExternalOutput").ap(),
    }
    with tile.TileContext(nc) as tc:
        _build(nc, tc, io)
    nc.compile()
    return nc


def make_in_maps(x, Wq, Wkv, Wo):
    import ml_dtypes

    bf16 = ml_dtypes.bfloat16
    cost, sincat = _rope_tables()
    in_maps = []
    for c in range(8):
        b, j = c // 4, c % 4
        xt = np.ascontiguousarray(x[b].T).reshape(NDB, 128, N)
        wq_c = np.ascontiguousarray(Wq[:, 256 * j:256 * (j + 1)]).reshape(
            NDB, 128, 256
        )
        wkv_c = np.ascontiguousarray(
            np.concatenate(
                [Wkv[:, 64 * j:64 * (j + 1)],
                 Wkv[:, 256 + 64 * j:256 + 64 * (j + 1)]],
                axis=1,
            )
        ).reshape(NDB, 128, 128)
        wo_c = np.ascontiguousarray(Wo[256 * j:256 * (j + 1), :]).reshape(
            2, 128, DIM
        )
        in_maps.append(
            {
                "xt": xt.astype(bf16),
                "wq": wq_c.astype(bf16),
                "wkv": wkv_c.astype(bf16),
                "wo": wo_c.astype(bf16),
                "cost": cost.astype(bf16),
                "sincat": sincat.astype(bf16),
            }
        )
    return in_maps


def _install_ntff_hook():
    """Register the axon NTFF profiling hook that this image's antenv lacks."""
    import types

    if "antenv.axon_hooks" in sys.modules:
        return
    try:
        sys.path.append("/root/.axon_site")
        from trn_agent_boot.trn_boot import _ntff_profile_via_ctypes

        hook = _ntff_profile_via_ctypes("/opt/axon/libaxon_pjrt.so")
    except Exception:
        hook = None
    finally:
        try:
            sys.path.remove("/root/.axon_site")
        except ValueError:
            pass
    mod = types.ModuleType("antenv.axon_hooks")
    mod.get_axon_ntff_profile_hook = lambda: hook
    mod.set_axon_ntff_profile_hook = lambda h: None
    sys.modules["antenv.axon_hooks"] = mod
    # artifact upload needs bucket credentials this container lacks
    import concourse.bass_utils as bu

    bu.upload_artifacts = lambda tmpdir: "local://" + str(tmpdir)


def kernel(x, Wq, Wkv, Wo, bo):
    from concourse.bass_utils import run_bass_kernel_spmd

    _install_ntff_hook()
    nc = _program()
    in_maps = make_in_maps(x, Wq, Wkv, Wo)
    trace = bool(os.environ.get("KERNEL_TRACE"))
    res = run_bass_kernel_spmd(
        nc, in_maps, list(range(8)), trace=trace
    )
    LAST_RESULTS["res"] = res
    full = np.zeros((B, N, DIM), np.float32)
    for c in range(8):
        full[c // 4] += res.results[c]["out"].astype(np.float32)
    full += bo.astype(np.float32)
    return full

